# revision 3
# baseline (speedup 1.0000x reference)
"""GTCN block (GCN -> temporal conv -> BN -> ReLU -> residual) on 8 trn2 cores.

Sharding: data-parallel over nodes. Each core takes 30000 nodes = 4 complete
(n, m) skeleton samples; adjacency / GCN / TCN params replicated.

Per-core dataflow (all layouts chosen so no op ever crosses partitions):
  h [125p x 240blk x 64] f32 (resident, reused for the residual)
  MM-A  : lhsT=h_block[125,64], rhs=blockdiag(An x5)[125,125] -> (A~ h)^T [64c, 125n]
  MM-B  : lhsT=[W|W][64,128],  rhs=(A~ h)^T bf16              -> y^T dup [128, 500]
  ACT   : relu(y+b) -> zT[0:64, t] and zT[64:128, t-25] (t+1 shifted copy, bf16)
  conv  : 5 accumulating matmuls, lhsT=[W2j;W2j+1 | dup][128,128], rhs=zT window
  ACT   : relu(s*conv+d) -> vt [128, 2x125] bf16 (two 125-blocks stacked)
  MM-T  : PE transpose vt -> [125n, 128c-pair] psum
  DVE   : psum + h -> out tile f32 -> DMA
"""

import numpy as np
import ml_dtypes

N, M, T, V, C = 16, 2, 300, 25, 64
KT, PAD = 9, 4
BN_EPS = 1e-5
NCORES = 8
LPC = N * M * T * V // NCORES   # 30000 nodes per core
BLK = 5 * V                     # 125 nodes per block (5 graphs)
GPC = LPC // BLK                # 240 blocks per core
SAMP = M * T * V // M           # 7500 nodes per (n,m) sample
SPC = LPC // SAMP               # 4 samples per core
SPB = SAMP // BLK               # 60 blocks per sample
GRP = 500                       # nodes per group (4 blocks)
NGRP = LPC // GRP               # 60 groups per core
GPS = SAMP // GRP               # 15 groups per sample
ZPAD = PAD * V                  # 100 zero columns per sample edge
SW = ZPAD + SAMP + ZPAD         # 7700 padded sample width in zT
BF16 = ml_dtypes.bfloat16

_CACHE = {}


def _split_multi_waits(nc, max_waits=1):
    """walrus in this toolchain rejects >1 sem wait per instruction
    ("Too many sync wait commands"); move excess waits onto preceding
    same-engine nops."""
    import concourse.mybir as mybir

    ctr = 0
    for f in nc.m.functions:
        for bb in f.blocks:
            out = []
            for ins in bb.instructions:
                si = ins.sync_info
                if si is not None and len(si.on_wait) > max_waits:
                    waits = list(si.on_wait)
                    keep = waits[len(waits) - max_waits:]
                    rest = waits[: len(waits) - max_waits]
                    for i in range(0, len(rest), max_waits):
                        ctr += 1
                        out.append(
                            mybir.InstNoOp(
                                name=f"I-wsplit-{ctr}",
                                engine=ins.engine,
                                sync_info=mybir.SyncInfo(
                                    on_wait=rest[i:i + max_waits], on_update=[]
                                ),
                            )
                        )
                    si.on_wait = keep
                out.append(ins)
            bb.instructions = out


def _build_program():
    import concourse.bass as bass
    import concourse.mybir as mybir
    import concourse.tile as tile

    f32 = mybir.dt.float32
    bf16 = mybir.dt.bfloat16
    RELU = mybir.ActivationFunctionType.Relu

    nc = bass.Bass()
    h_t = nc.declare_dram_parameter("h", [LPC, C], f32, isOutput=False)
    bd_t = nc.declare_dram_parameter("bd", [BLK, BLK], f32, isOutput=False)
    gw_t = nc.declare_dram_parameter("gw2", [C, 2 * C], bf16, isOutput=False)
    gb_t = nc.declare_dram_parameter("gb2", [2 * C, 1], f32, isOutput=False)
    wp_t = nc.declare_dram_parameter("wp2", [2 * C, 5, 2 * C], bf16, isOutput=False)
    bns_t = nc.declare_dram_parameter("bns2", [2 * C, 1], f32, isOutput=False)
    bnd_t = nc.declare_dram_parameter("bnd2", [2 * C, 1], f32, isOutput=False)
    id_t = nc.declare_dram_parameter("ident", [2 * C, 2 * C], bf16, isOutput=False)
    out_t = nc.declare_dram_parameter("out", [LPC, C], f32, isOutput=True)

    h_r = h_t[:, :].rearrange("(g p) c -> p g c", p=BLK)
    out_r = out_t[:, :].rearrange("(g p) c -> p g c", p=BLK)

    with tile.TileContext(nc) as tc:
        import contextlib

        with contextlib.ExitStack() as ctx:
            persist = ctx.enter_context(tc.tile_pool(name="persist", bufs=1))
            abufp = ctx.enter_context(tc.tile_pool(name="abuf", bufs=3))
            vtp = ctx.enter_context(tc.tile_pool(name="vt", bufs=3))
            ogp = ctx.enter_context(tc.tile_pool(name="og", bufs=3))
            pa = ctx.enter_context(tc.tile_pool(name="pa", bufs=2, space="PSUM"))
            py = ctx.enter_context(tc.tile_pool(name="py", bufs=2, space="PSUM"))
            pc = ctx.enter_context(tc.tile_pool(name="pc", bufs=2, space="PSUM"))
            pt = ctx.enter_context(tc.tile_pool(name="pt", bufs=2, space="PSUM"))

            # ---- persistent state ----
            bd_sb = persist.tile([BLK, BLK], f32)
            nc.sync.dma_start(out=bd_sb, in_=bd_t[:, :])
            gw_sb = persist.tile([C, 2 * C], bf16)
            nc.sync.dma_start(out=gw_sb, in_=gw_t[:, :])
            gb_sb = persist.tile([2 * C, 1], f32)
            nc.sync.dma_start(out=gb_sb, in_=gb_t[:, :])
            wp_sb = persist.tile([2 * C, 5, 2 * C], bf16)
            nc.sync.dma_start(out=wp_sb, in_=wp_t[:, :, :])
            bns_sb = persist.tile([2 * C, 1], f32)
            nc.sync.dma_start(out=bns_sb, in_=bns_t[:, :])
            bnd_sb = persist.tile([2 * C, 1], f32)
            nc.sync.dma_start(out=bnd_sb, in_=bnd_t[:, :])
            id_sb = persist.tile([2 * C, 2 * C], bf16)
            nc.sync.dma_start(out=id_sb, in_=id_t[:, :])

            hbuf = []
            for s in range(SPC):
                hb = persist.tile([BLK, SPB, C], f32, tag=f"hbuf{s}")
                nc.sync.dma_start(out=hb, in_=h_r[:, s * SPB:(s + 1) * SPB, :])
                hbuf.append(hb)

            zt = persist.tile([2 * C, SPC * SW], bf16)
            for s in range(SPC):
                b = s * SW
                nc.vector.memset(zt[:, b:b + ZPAD], 0.0)
                nc.vector.memset(zt[:, b + ZPAD + SAMP:b + SW], 0.0)
                nc.vector.memset(zt[64:128, b + ZPAD + SAMP - V:b + ZPAD + SAMP], 0.0)

            # ---- pipeline over 500-node groups ----
            def g_stage(j):
                s, l = divmod(j, GPS)
                zoff = s * SW + ZPAD + l * GRP
                g0 = (j * 4) % SPB  # block index within sample
                psa = pa.tile([C, GRP], f32)
                for i in range(4):
                    nc.tensor.matmul(
                        psa[:, i * BLK:(i + 1) * BLK],
                        lhsT=hbuf[s][:, g0 + i, :],
                        rhs=bd_sb,
                        start=True, stop=True,
                    )
                ab = abufp.tile([C, GRP], bf16)
                nc.vector.tensor_copy(ab, psa)
                psy = py.tile([2 * C, GRP], f32)
                nc.tensor.matmul(psy, lhsT=gw_sb, rhs=ab, start=True, stop=True)
                nc.scalar.activation(
                    out=zt[0:C, zoff:zoff + GRP], in_=psy[0:C, :],
                    func=RELU, bias=gb_sb[0:C, :],
                )
                nc.scalar.activation(
                    out=zt[C:2 * C, zoff - V:zoff + GRP - V], in_=psy[C:2 * C, :],
                    func=RELU, bias=gb_sb[C:2 * C, :],
                )

            def c_stage(j):
                s, l = divmod(j, GPS)
                zoff = s * SW + ZPAD + l * GRP
                g0 = (j * 4) % SPB
                psc = pc.tile([2 * C, GRP], f32)
                for jj in range(5):
                    nc.tensor.matmul(
                        psc,
                        lhsT=wp_sb[:, jj, :],
                        rhs=zt[:, zoff + (2 * jj - 4) * V: zoff + (2 * jj - 4) * V + GRP],
                        start=(jj == 0), stop=(jj == 4),
                    )
                vt = vtp.tile([2 * C, 2, BLK], bf16)
                pcv = psc.rearrange("p (b x) -> p b x", b=2)
                nc.scalar.activation(
                    out=vt[0:C, :, :], in_=pcv[0:C, :, 0:BLK],
                    func=RELU, bias=bnd_sb[0:C, :], scale=bns_sb[0:C, :],
                )
                nc.scalar.activation(
                    out=vt[C:2 * C, :, :], in_=pcv[C:2 * C, :, BLK:2 * BLK],
                    func=RELU, bias=bnd_sb[C:2 * C, :], scale=bns_sb[C:2 * C, :],
                )
                og = ogp.tile([BLK, 4, C], f32)
                for q in range(2):
                    pst = pt.tile([BLK, 2 * C], bf16)
                    nc.tensor.transpose(pst, in_=vt[:, q, :], identity=id_sb)
                    nc.vector.tensor_add(
                        og[:, 2 * q:2 * q + 2, :].rearrange("p a b -> p (a b)"),
                        pst,
                        hbuf[s][:, g0 + 2 * q:g0 + 2 * q + 2, :].rearrange(
                            "p a b -> p (a b)"),
                    )
                nc.sync.dma_start(out=out_r[:, j * 4:j * 4 + 4, :], in_=og)

            g_stage(0)
            for j in range(1, NGRP):
                g_stage(j)
                c_stage(j - 1)
            c_stage(NGRP - 1)

    _split_multi_waits(nc)
    return nc


def _host_prep(adj, gcn_w, gcn_b, conv_w, conv_b, bn_gamma, bn_beta, bn_mean, bn_var):
    adj = np.asarray(adj, np.float64)
    norm = adj.sum(axis=1) ** -0.5
    an = (norm[:, None] * adj * norm[None, :]).astype(np.float32)
    bd = np.kron(np.eye(5, dtype=np.float32), an)              # [125,125]

    w = np.asarray(gcn_w, np.float32)                          # [cin, cout]
    gw2 = np.concatenate([w, w], axis=1).astype(BF16)          # [64,128]
    gb2 = np.tile(np.asarray(gcn_b, np.float32), 2)[:, None]   # [128,1]

    cw = np.asarray(conv_w, np.float32)[:, :, :, 0]            # [cout, cin, KT]
    wp2 = np.zeros((2 * C, 5, 2 * C), np.float32)
    for jj in range(5):
        wp2[0:C, jj, 0:C] = cw[:, :, 2 * jj].T
        wp2[0:C, jj, C:2 * C] = cw[:, :, 2 * jj].T
        if 2 * jj + 1 < KT:
            wp2[C:2 * C, jj, 0:C] = cw[:, :, 2 * jj + 1].T
            wp2[C:2 * C, jj, C:2 * C] = cw[:, :, 2 * jj + 1].T
    wp2 = wp2.astype(BF16)

    s = np.asarray(bn_gamma, np.float32) / np.sqrt(np.asarray(bn_var, np.float32) + BN_EPS)
    d = (np.asarray(conv_b, np.float32) - np.asarray(bn_mean, np.float32)) * s \
        + np.asarray(bn_beta, np.float32)
    bns2 = np.tile(s, 2)[:, None].astype(np.float32)
    bnd2 = np.tile(d, 2)[:, None].astype(np.float32)
    ident = np.eye(2 * C, dtype=BF16)
    return dict(bd=bd, gw2=gw2, gb2=gb2, wp2=wp2, bns2=bns2, bnd2=bnd2, ident=ident)


def kernel(h, adj, gcn_w, gcn_b, conv_w, conv_b, bn_gamma, bn_beta, bn_mean, bn_var):
    from concourse.bass_utils import run_bass_kernel_spmd

    h = np.ascontiguousarray(np.asarray(h, np.float32))
    weights = _host_prep(adj, gcn_w, gcn_b, conv_w, conv_b,
                         bn_gamma, bn_beta, bn_mean, bn_var)

    if "nc" not in _CACHE:
        _CACHE["nc"] = _build_program()
    nc = _CACHE["nc"]

    in_maps = []
    for i in range(NCORES):
        m = dict(weights)
        m["h"] = h[i * LPC:(i + 1) * LPC]
        in_maps.append(m)

    res = run_bass_kernel_spmd(nc, in_maps, core_ids=list(range(NCORES)))
    out = np.concatenate([res.results[i]["out"] for i in range(NCORES)], axis=0)
    return out.reshape(N, M, T, V, C)


# revision 36
# speedup vs baseline: 1.4171x; 1.4171x over previous
"""GTCN block (GCN -> temporal conv -> BN -> ReLU -> residual) on 8 trn2 cores.

Sharding: data-parallel over nodes. Each core takes 30000 nodes = 4 complete
(n, m) skeleton samples; adjacency / GCN / TCN params replicated.

Per-core dataflow (channel-major through the middle, node-PAIR major at the
edges so every HBM descriptor is a 512B line-rate run):
  hres [125p x 120pg x (2,64)] bf16: partition x of pair-group g2 holds nodes
        (250*g2+2x, +1) -- feeds both the aggregation and the residual.
  MM-A : even/odd split blockdiag adjacency, 2 accumulating matmuls per
         250-node pair-group -> (A~ h)^T [64c, 250n] channel-major psum
  MM-B : lhsT=W[64,64], rhs=(A~ h)^T bf16 -> y^T [64, 500]
  ACT  : relu(y+b) -> zT[0:64, t]; SBUF->SBUF DMA -> zT[64:128, t-25]
  conv : 5 accumulating matmuls, lhsT=[W2j;W2j+1]x2 dup [128,128], rhs=zT win
  ACT  : relu(s*conv+d), stride-2 reads (even/odd nodes) -> vt [128, 250]
  MM-T : PE transpose vt[:,125q:+125] -> pst[125 pair, 128=(2 nodes x 64c)]
  DVE  : pst + hres -> og staging; one SWDGE out-DMA per 4 groups (512B descs)
"""

import numpy as np
import ml_dtypes

N, M, T, V, C = 16, 2, 300, 25, 64
KT, PAD = 9, 4
BN_EPS = 1e-5
NCORES = 8
LPC = N * M * T * V // NCORES   # 30000 nodes per core
BLK = 5 * V                     # 125 nodes per block (5 graphs)
PG = 2 * BLK                    # 250 nodes per pair-group (10 graphs)
NPG = LPC // PG                 # 120 pair-groups per core
SAMP = T * V                    # 7500 nodes per (n,m) sample
SPC = LPC // SAMP               # 4 samples per core
PGS = SAMP // PG                # 30 pair-groups per sample
GRP = 500                       # nodes per group (2 pair-groups)
NGRP = LPC // GRP               # 60 groups per core
GPS = SAMP // GRP               # 15 groups per sample
ZPAD = PAD * V                  # 100 zero columns per sample edge
SW = ZPAD + SAMP + ZPAD         # 7700 padded sample width in zT
OB = 4                          # groups per out-DMA batch
BF16 = ml_dtypes.bfloat16

_CACHE = {}


def _split_multi_waits(nc, max_waits=1):
    """walrus in this toolchain rejects >1 sem wait per instruction
    ("Too many sync wait commands"); move excess waits onto preceding
    same-engine nops."""
    import concourse.mybir as mybir

    ctr = 0
    for f in nc.m.functions:
        for bb in f.blocks:
            out = []
            for ins in bb.instructions:
                si = ins.sync_info
                if si is not None and len(si.on_wait) > max_waits:
                    waits = list(si.on_wait)
                    keep = waits[len(waits) - max_waits:]
                    rest = waits[: len(waits) - max_waits]
                    for i in range(0, len(rest), max_waits):
                        ctr += 1
                        out.append(
                            mybir.InstNoOp(
                                name=f"I-wsplit-{ctr}",
                                engine=ins.engine,
                                sync_info=mybir.SyncInfo(
                                    on_wait=rest[i:i + max_waits], on_update=[]
                                ),
                            )
                        )
                    si.on_wait = keep
                out.append(ins)
            bb.instructions = out


def _build_program():
    import concourse.bass as bass
    import concourse.mybir as mybir
    import concourse.tile as tile

    f32 = mybir.dt.float32
    bf16 = mybir.dt.bfloat16
    RELU = mybir.ActivationFunctionType.Relu

    nc = bass.Bass()
    h_t = nc.declare_dram_parameter("h", [LPC, C], f32, isOutput=False)
    bd_t = nc.declare_dram_parameter("bdeo", [BLK, 2, PG], bf16, isOutput=False)
    gw_t = nc.declare_dram_parameter("gw", [2 * C, C], bf16, isOutput=False)
    gb_t = nc.declare_dram_parameter("gb", [C, 1], f32, isOutput=False)
    wp_t = nc.declare_dram_parameter("wp2", [2 * C, 5, 2 * C], bf16, isOutput=False)
    bns_t = nc.declare_dram_parameter("bns2", [2 * C, 1], f32, isOutput=False)
    bnd_t = nc.declare_dram_parameter("bnd2", [2 * C, 1], f32, isOutput=False)
    id_t = nc.declare_dram_parameter("ident", [2 * C, 2 * C], bf16, isOutput=False)
    out_t = nc.declare_dram_parameter("out", [LPC, C], f32, isOutput=True)

    # pair-major views: partition x <-> nodes (250*g2 + 2x, +1)
    h_rp = h_t[:, :].rearrange("(g2 p two) c -> p g2 (two c)", p=BLK, two=2)
    out_rp = out_t[:, :].rearrange("(g2 p two) c -> p g2 (two c)", p=BLK, two=2)

    with tile.TileContext(nc) as tc:
        import contextlib

        with contextlib.ExitStack() as ctx:
            persist = ctx.enter_context(tc.tile_pool(name="persist", bufs=1))
            abufp = ctx.enter_context(tc.tile_pool(name="abuf", bufs=6))
            vtp = ctx.enter_context(tc.tile_pool(name="vt", bufs=4))
            ogp = ctx.enter_context(tc.tile_pool(name="og", bufs=6))
            pa = ctx.enter_context(tc.tile_pool(name="pa", bufs=2, space="PSUM"))
            py = ctx.enter_context(tc.tile_pool(name="py", bufs=2, space="PSUM"))
            pc = ctx.enter_context(tc.tile_pool(name="pc", bufs=2, space="PSUM"))
            pt = ctx.enter_context(tc.tile_pool(name="pt", bufs=2, space="PSUM"))

            # ---- persistent state ----
            bd_sb = persist.tile([BLK, 2, PG], bf16)
            nc.sync.dma_start(out=bd_sb, in_=bd_t[:, :, :])
            gwd_sb = persist.tile([2 * C, C], bf16)
            nc.sync.dma_start(out=gwd_sb, in_=gw_t[:, :])
            gb_sb = persist.tile([C, 1], f32)
            nc.sync.dma_start(out=gb_sb, in_=gb_t[:, :])
            wp_sb = persist.tile([2 * C, 5, 2 * C], bf16)
            nc.sync.dma_start(out=wp_sb, in_=wp_t[:, :, :])
            bns_sb = persist.tile([2 * C, 1], f32)
            nc.sync.dma_start(out=bns_sb, in_=bns_t[:, :])
            bnd_sb = persist.tile([2 * C, 1], f32)
            nc.sync.dma_start(out=bnd_sb, in_=bnd_t[:, :])
            id_sb = persist.tile([2 * C, 2 * C], bf16)
            nc.sync.dma_start(out=id_sb, in_=id_t[:, :])

            # h resident in bf16 pair-major (cast during SWDGE DMA); 512B/desc
            hres = persist.tile([BLK, NPG, 2, C], bf16)
            hchunks = [0, 2, 6, 14, 30, 45, 60, 75, 90, 105, NPG]
            for q in range(len(hchunks) - 1):
                nc.gpsimd.dma_start(
                    out=hres[:, hchunks[q]:hchunks[q + 1], :, :].rearrange(
                        "p g a b -> p g (a b)"),
                    in_=h_rp[:, hchunks[q]:hchunks[q + 1], :])

            zt = persist.tile([2 * C, SPC * SW], bf16)
            for s in range(SPC):
                b = s * SW
                nc.vector.memset(zt[:, b:b + ZPAD], 0.0)
                nc.vector.memset(zt[:, b + ZPAD + SAMP:b + SW], 0.0)
                nc.vector.memset(zt[64:128, b + ZPAD + SAMP - V:b + ZPAD + SAMP], 0.0)

            # ---- pipeline over 500-node groups ----
            abufs = {}
            ogs = {}

            def a_stage(j):
                psa = pa.tile([C, GRP], f32)
                for i in range(2):
                    g2 = 2 * j + i
                    nc.tensor.matmul(
                        psa[:, i * PG:(i + 1) * PG],
                        lhsT=hres[:, g2, 0, :], rhs=bd_sb[:, 0, :],
                        start=True, stop=False,
                    )
                    nc.tensor.matmul(
                        psa[:, i * PG:(i + 1) * PG],
                        lhsT=hres[:, g2, 1, :], rhs=bd_sb[:, 1, :],
                        start=False, stop=True,
                    )
                ab = abufp.tile([C, GRP], bf16)
                nc.vector.tensor_copy(ab, psa)
                abufs[j] = ab

            def b_stage(j):
                s, l = divmod(j, GPS)
                zoff = s * SW + ZPAD + l * GRP
                psy = py.tile([C, GRP], f32)
                nc.tensor.matmul(psy, lhsT=gwd_sb[0:C, :], rhs=abufs.pop(j),
                                 start=True, stop=True)
                nc.scalar.activation(
                    out=zt[0:C, zoff:zoff + GRP], in_=psy,
                    func=RELU, bias=gb_sb,
                )
                # t+1-shifted copy for the bottom partition half: cross-partition,
                # so an SBUF->SBUF DMA (reads the relu'd top half). Issued from
                # ScalarE's HWDGE ring so it never queues behind bulk h/out DMAs.
                nc.scalar.dma_start(
                    out=zt[C:2 * C, zoff - V:zoff + GRP - V],
                    in_=zt[0:C, zoff:zoff + GRP],
                )

            pscs = {}

            def c_conv(j):
                s, l = divmod(j, GPS)
                zoff = s * SW + ZPAD + l * GRP
                psc = pc.tile([2 * C, GRP], f32, name="psc", tag="psc")
                for jj in range(5):
                    nc.tensor.matmul(
                        psc,
                        lhsT=wp_sb[:, jj, :],
                        rhs=zt[:, zoff + (2 * jj - 4) * V: zoff + (2 * jj - 4) * V + GRP],
                        start=(jj == 0), stop=(jj == 4),
                    )
                pscs[j] = psc

            def c_post(j):
                psc = pscs.pop(j)
                # even nodes -> vt top half, odd nodes -> vt bottom half
                # (psc bottom partition half duplicates the top, keeping the
                # stride-2 reads partition-aligned)
                vt = vtp.tile([2 * C, PG], bf16)
                pcv = psc.rearrange("p (x two) -> p x two", two=2)
                nc.scalar.activation(
                    out=vt[0:C, :], in_=pcv[0:C, :, 0],
                    func=RELU, bias=bnd_sb[0:C, :], scale=bns_sb[0:C, :],
                )
                nc.scalar.activation(
                    out=vt[C:2 * C, :], in_=pcv[C:2 * C, :, 1],
                    func=RELU, bias=bnd_sb[C:2 * C, :], scale=bns_sb[C:2 * C, :],
                )
                jb, jo = divmod(j, OB)
                if jo == 0:
                    ogs[jb] = ogp.tile([BLK, 2 * OB, 2 * C], f32, name="og", tag="og")
                og = ogs[jb]
                for q in range(2):
                    pst = pt.tile([BLK, 2 * C], bf16)
                    nc.tensor.transpose(pst, in_=vt[:, q * BLK:(q + 1) * BLK],
                                        identity=id_sb)
                    nc.vector.tensor_add(
                        og[:, jo * 2 + q, :],
                        pst,
                        hres[:, 2 * j + q, :, :].rearrange("p a b -> p (a b)"),
                    )
                last = jb == NGRP // OB - 1
                if last:
                    # flush per group at the end so the kernel-tail drain only
                    # waits on one tiny DMA
                    nc.gpsimd.dma_start(
                        out=out_rp[:, jb * 2 * OB + 2 * jo:jb * 2 * OB + 2 * jo + 2, :],
                        in_=og[:, 2 * jo:2 * jo + 2, :])
                    if jo == OB - 1:
                        ogs.pop(jb)
                elif jo == OB // 2 - 1:
                    # flush the first half-batch early
                    nc.gpsimd.dma_start(
                        out=out_rp[:, jb * 2 * OB:jb * 2 * OB + OB, :],
                        in_=og[:, 0:OB, :])
                elif jo == OB - 1:
                    nc.gpsimd.dma_start(
                        out=out_rp[:, jb * 2 * OB + OB:(jb + 1) * 2 * OB, :],
                        in_=ogs.pop(jb)[:, OB:2 * OB, :])

            a_stage(0)
            a_stage(1)
            b_stage(0)
            for jj in range(2, 8):
                a_stage(jj)
                b_stage(jj - 1)
            for j in range(NGRP):
                c_conv(j)
                c_post(j)
                if j + 8 < NGRP:
                    a_stage(j + 8)
                if j + 7 < NGRP:
                    b_stage(j + 7)

    _split_multi_waits(nc)
    return nc


def _host_prep(adj, gcn_w, gcn_b, conv_w, conv_b, bn_gamma, bn_beta, bn_mean, bn_var):
    adj = np.asarray(adj, np.float64)
    norm = adj.sum(axis=1) ** -0.5
    an = (norm[:, None] * adj * norm[None, :]).astype(np.float32)
    a10 = np.kron(np.eye(10, dtype=np.float32), an)            # [250,250]
    bdeo = np.stack([a10[0::2, :], a10[1::2, :]], axis=1).astype(BF16)  # [125,2,250]

    gw1 = np.asarray(gcn_w, np.float32)
    gw = np.concatenate([gw1, gw1], axis=0).astype(BF16)       # [128, 64] dup
    gb = np.asarray(gcn_b, np.float32)[:, None]                # [64,1]

    cw = np.asarray(conv_w, np.float32)[:, :, :, 0]            # [cout, cin, KT]
    wp2 = np.zeros((2 * C, 5, 2 * C), np.float32)
    for jj in range(5):
        wp2[0:C, jj, 0:C] = cw[:, :, 2 * jj].T
        wp2[0:C, jj, C:2 * C] = cw[:, :, 2 * jj].T
        if 2 * jj + 1 < KT:
            wp2[C:2 * C, jj, 0:C] = cw[:, :, 2 * jj + 1].T
            wp2[C:2 * C, jj, C:2 * C] = cw[:, :, 2 * jj + 1].T
    wp2 = wp2.astype(BF16)

    s = np.asarray(bn_gamma, np.float32) / np.sqrt(np.asarray(bn_var, np.float32) + BN_EPS)
    d = (np.asarray(conv_b, np.float32) - np.asarray(bn_mean, np.float32)) * s \
        + np.asarray(bn_beta, np.float32)
    bns2 = np.tile(s, 2)[:, None].astype(np.float32)
    bnd2 = np.tile(d, 2)[:, None].astype(np.float32)
    ident = np.eye(2 * C, dtype=BF16)
    return dict(bdeo=bdeo, gw=gw, gb=gb, wp2=wp2, bns2=bns2, bnd2=bnd2, ident=ident)


def kernel(h, adj, gcn_w, gcn_b, conv_w, conv_b, bn_gamma, bn_beta, bn_mean, bn_var):
    from concourse.bass_utils import run_bass_kernel_spmd

    h = np.ascontiguousarray(np.asarray(h, np.float32))
    weights = _host_prep(adj, gcn_w, gcn_b, conv_w, conv_b,
                         bn_gamma, bn_beta, bn_mean, bn_var)

    if "nc" not in _CACHE:
        _CACHE["nc"] = _build_program()
    nc = _CACHE["nc"]

    in_maps = []
    for i in range(NCORES):
        m = dict(weights)
        m["h"] = h[i * LPC:(i + 1) * LPC]
        in_maps.append(m)

    res = run_bass_kernel_spmd(nc, in_maps, core_ids=list(range(NCORES)))
    out = np.concatenate([res.results[i]["out"] for i in range(NCORES)], axis=0)
    return out.reshape(N, M, T, V, C)


# revision 37
# speedup vs baseline: 1.4318x; 1.0104x over previous
"""GTCN block (GCN -> temporal conv -> BN -> ReLU -> residual) on 8 trn2 cores.

Sharding: data-parallel over nodes. Each core takes 30000 nodes = 4 complete
(n, m) skeleton samples; adjacency / GCN / TCN params replicated.

Per-core dataflow (channel-major through the middle, node-PAIR major at the
edges so every HBM descriptor is a 512B line-rate run):
  hres [125p x 120pg x (2,64)] bf16: partition x of pair-group g2 holds nodes
        (250*g2+2x, +1) -- feeds both the aggregation and the residual.
  MM-A : even/odd split blockdiag adjacency, 2 accumulating matmuls per
         250-node pair-group -> (A~ h)^T [64c, 250n] channel-major psum
  MM-B : lhsT=W[64,64], rhs=(A~ h)^T bf16 -> y^T [64, 500]
  ACT  : relu(y+b) -> zT[0:64, t]; SBUF->SBUF DMA -> zT[64:128, t-25]
  conv : 5 accumulating matmuls, lhsT=[W2j;W2j+1]x2 dup [128,128], rhs=zT win
  ACT  : relu(s*conv+d), stride-2 reads (even/odd nodes) -> vt [128, 250]
  MM-T : PE transpose vt[:,125q:+125] -> pst[125 pair, 128=(2 nodes x 64c)]
  DVE  : pst + hres -> og staging; one SWDGE out-DMA per 4 groups (512B descs)
"""

import numpy as np
import ml_dtypes

N, M, T, V, C = 16, 2, 300, 25, 64
KT, PAD = 9, 4
BN_EPS = 1e-5
NCORES = 8
LPC = N * M * T * V // NCORES   # 30000 nodes per core
BLK = 5 * V                     # 125 nodes per block (5 graphs)
PG = 2 * BLK                    # 250 nodes per pair-group (10 graphs)
NPG = LPC // PG                 # 120 pair-groups per core
SAMP = T * V                    # 7500 nodes per (n,m) sample
SPC = LPC // SAMP               # 4 samples per core
PGS = SAMP // PG                # 30 pair-groups per sample
GRP = 500                       # nodes per group (2 pair-groups)
NGRP = LPC // GRP               # 60 groups per core
GPS = SAMP // GRP               # 15 groups per sample
ZPAD = PAD * V                  # 100 zero columns per sample edge
SW = ZPAD + SAMP + ZPAD         # 7700 padded sample width in zT
OB = 4                          # groups per out-DMA batch
BF16 = ml_dtypes.bfloat16

_CACHE = {}


def _split_multi_waits(nc, max_waits=1):
    """walrus in this toolchain rejects >1 sem wait per instruction
    ("Too many sync wait commands"); move excess waits onto preceding
    same-engine nops."""
    import concourse.mybir as mybir

    ctr = 0
    for f in nc.m.functions:
        for bb in f.blocks:
            out = []
            for ins in bb.instructions:
                si = ins.sync_info
                if si is not None and len(si.on_wait) > max_waits:
                    waits = list(si.on_wait)
                    keep = waits[len(waits) - max_waits:]
                    rest = waits[: len(waits) - max_waits]
                    for i in range(0, len(rest), max_waits):
                        ctr += 1
                        out.append(
                            mybir.InstNoOp(
                                name=f"I-wsplit-{ctr}",
                                engine=ins.engine,
                                sync_info=mybir.SyncInfo(
                                    on_wait=rest[i:i + max_waits], on_update=[]
                                ),
                            )
                        )
                    si.on_wait = keep
                out.append(ins)
            bb.instructions = out


def _build_program():
    import concourse.bass as bass
    import concourse.mybir as mybir
    import concourse.tile as tile

    f32 = mybir.dt.float32
    bf16 = mybir.dt.bfloat16
    RELU = mybir.ActivationFunctionType.Relu

    nc = bass.Bass()
    h_t = nc.declare_dram_parameter("h", [LPC, C], f32, isOutput=False)
    bd_t = nc.declare_dram_parameter("bdeo", [BLK, 2, PG], bf16, isOutput=False)
    gw_t = nc.declare_dram_parameter("gw", [2 * C, C], bf16, isOutput=False)
    gb_t = nc.declare_dram_parameter("gb", [C, 1], f32, isOutput=False)
    wp_t = nc.declare_dram_parameter("wp2", [2 * C, 5, 2 * C], bf16, isOutput=False)
    bns_t = nc.declare_dram_parameter("bns2", [2 * C, 1], f32, isOutput=False)
    bnd_t = nc.declare_dram_parameter("bnd2", [2 * C, 1], f32, isOutput=False)
    id_t = nc.declare_dram_parameter("ident", [2 * C, 2 * C], bf16, isOutput=False)
    out_t = nc.declare_dram_parameter("out", [LPC, C], f32, isOutput=True)

    # pair-major views: partition x <-> nodes (250*g2 + 2x, +1)
    h_rp = h_t[:, :].rearrange("(g2 p two) c -> p g2 (two c)", p=BLK, two=2)
    out_rp = out_t[:, :].rearrange("(g2 p two) c -> p g2 (two c)", p=BLK, two=2)

    with tile.TileContext(nc) as tc:
        import contextlib

        with contextlib.ExitStack() as ctx:
            persist = ctx.enter_context(tc.tile_pool(name="persist", bufs=1))
            abufp = ctx.enter_context(tc.tile_pool(name="abuf", bufs=6))
            vtp = ctx.enter_context(tc.tile_pool(name="vt", bufs=4))
            ogp = ctx.enter_context(tc.tile_pool(name="og", bufs=6))
            pa = ctx.enter_context(tc.tile_pool(name="pa", bufs=2, space="PSUM"))
            py = ctx.enter_context(tc.tile_pool(name="py", bufs=2, space="PSUM"))
            pc = ctx.enter_context(tc.tile_pool(name="pc", bufs=2, space="PSUM"))
            pt = ctx.enter_context(tc.tile_pool(name="pt", bufs=2, space="PSUM"))

            # ---- persistent state ----
            bd_sb = persist.tile([BLK, 2, PG], bf16)
            nc.sync.dma_start(out=bd_sb, in_=bd_t[:, :, :])
            gwd_sb = persist.tile([2 * C, C], bf16)
            nc.sync.dma_start(out=gwd_sb, in_=gw_t[:, :])
            gb_sb = persist.tile([C, 1], f32)
            nc.sync.dma_start(out=gb_sb, in_=gb_t[:, :])
            wp_sb = persist.tile([2 * C, 5, 2 * C], bf16)
            nc.sync.dma_start(out=wp_sb, in_=wp_t[:, :, :])
            bns_sb = persist.tile([2 * C, 1], f32)
            nc.sync.dma_start(out=bns_sb, in_=bns_t[:, :])
            bnd_sb = persist.tile([2 * C, 1], f32)
            nc.sync.dma_start(out=bnd_sb, in_=bnd_t[:, :])
            id_sb = persist.tile([2 * C, 2 * C], bf16)
            nc.sync.dma_start(out=id_sb, in_=id_t[:, :])

            # h resident in bf16 pair-major (cast during SWDGE DMA); 512B/desc
            hres = persist.tile([BLK, NPG, 2, C], bf16)
            hchunks = [0, 1, 3, 7, 15, 25, 35, 45, 55, 65, 75, 85, 95, 105, NPG]
            for q in range(len(hchunks) - 1):
                nc.gpsimd.dma_start(
                    out=hres[:, hchunks[q]:hchunks[q + 1], :, :].rearrange(
                        "p g a b -> p g (a b)"),
                    in_=h_rp[:, hchunks[q]:hchunks[q + 1], :])

            zt = persist.tile([2 * C, SPC * SW], bf16)
            for s in range(SPC):
                b = s * SW
                nc.vector.memset(zt[:, b:b + ZPAD], 0.0)
                nc.vector.memset(zt[:, b + ZPAD + SAMP:b + SW], 0.0)
                nc.vector.memset(zt[64:128, b + ZPAD + SAMP - V:b + ZPAD + SAMP], 0.0)

            # ---- pipeline over 500-node groups ----
            abufs = {}
            ogs = {}

            def a_stage(j):
                psa = pa.tile([C, GRP], f32)
                for i in range(2):
                    g2 = 2 * j + i
                    nc.tensor.matmul(
                        psa[:, i * PG:(i + 1) * PG],
                        lhsT=hres[:, g2, 0, :], rhs=bd_sb[:, 0, :],
                        start=True, stop=False,
                    )
                    nc.tensor.matmul(
                        psa[:, i * PG:(i + 1) * PG],
                        lhsT=hres[:, g2, 1, :], rhs=bd_sb[:, 1, :],
                        start=False, stop=True,
                    )
                ab = abufp.tile([C, GRP], bf16)
                nc.vector.tensor_copy(ab, psa)
                abufs[j] = ab

            def b_stage(j):
                s, l = divmod(j, GPS)
                zoff = s * SW + ZPAD + l * GRP
                psy = py.tile([C, GRP], f32)
                nc.tensor.matmul(psy, lhsT=gwd_sb[0:C, :], rhs=abufs.pop(j),
                                 start=True, stop=True)
                nc.scalar.activation(
                    out=zt[0:C, zoff:zoff + GRP], in_=psy,
                    func=RELU, bias=gb_sb,
                )
                # t+1-shifted copy for the bottom partition half: cross-partition,
                # so an SBUF->SBUF DMA (reads the relu'd top half). Issued from
                # ScalarE's HWDGE ring so it never queues behind bulk h/out DMAs.
                nc.scalar.dma_start(
                    out=zt[C:2 * C, zoff - V:zoff + GRP - V],
                    in_=zt[0:C, zoff:zoff + GRP],
                )

            pscs = {}

            def c_conv(j):
                s, l = divmod(j, GPS)
                zoff = s * SW + ZPAD + l * GRP
                psc = pc.tile([2 * C, GRP], f32, name="psc", tag="psc")
                for jj in range(5):
                    nc.tensor.matmul(
                        psc,
                        lhsT=wp_sb[:, jj, :],
                        rhs=zt[:, zoff + (2 * jj - 4) * V: zoff + (2 * jj - 4) * V + GRP],
                        start=(jj == 0), stop=(jj == 4),
                    )
                pscs[j] = psc

            def c_post(j):
                psc = pscs.pop(j)
                # even nodes -> vt top half, odd nodes -> vt bottom half
                # (psc bottom partition half duplicates the top, keeping the
                # stride-2 reads partition-aligned)
                vt = vtp.tile([2 * C, PG], bf16)
                pcv = psc.rearrange("p (x two) -> p x two", two=2)
                nc.scalar.activation(
                    out=vt[0:C, :], in_=pcv[0:C, :, 0],
                    func=RELU, bias=bnd_sb[0:C, :], scale=bns_sb[0:C, :],
                )
                nc.scalar.activation(
                    out=vt[C:2 * C, :], in_=pcv[C:2 * C, :, 1],
                    func=RELU, bias=bnd_sb[C:2 * C, :], scale=bns_sb[C:2 * C, :],
                )
                jb, jo = divmod(j, OB)
                if jo == 0:
                    ogs[jb] = ogp.tile([BLK, 2 * OB, 2 * C], f32, name="og", tag="og")
                og = ogs[jb]
                for q in range(2):
                    pst = pt.tile([BLK, 2 * C], bf16)
                    nc.tensor.transpose(pst, in_=vt[:, q * BLK:(q + 1) * BLK],
                                        identity=id_sb)
                    nc.vector.tensor_add(
                        og[:, jo * 2 + q, :],
                        pst,
                        hres[:, 2 * j + q, :, :].rearrange("p a b -> p (a b)"),
                    )
                last = jb == NGRP // OB - 1
                if last:
                    # flush per group at the end so the kernel-tail drain only
                    # waits on one tiny DMA
                    nc.gpsimd.dma_start(
                        out=out_rp[:, jb * 2 * OB + 2 * jo:jb * 2 * OB + 2 * jo + 2, :],
                        in_=og[:, 2 * jo:2 * jo + 2, :])
                    if jo == OB - 1:
                        ogs.pop(jb)
                elif jo == OB // 2 - 1:
                    # flush the first half-batch early
                    nc.gpsimd.dma_start(
                        out=out_rp[:, jb * 2 * OB:jb * 2 * OB + OB, :],
                        in_=og[:, 0:OB, :])
                elif jo == OB - 1:
                    nc.gpsimd.dma_start(
                        out=out_rp[:, jb * 2 * OB + OB:(jb + 1) * 2 * OB, :],
                        in_=ogs.pop(jb)[:, OB:2 * OB, :])

            a_stage(0)
            a_stage(1)
            b_stage(0)
            for jj in range(2, 8):
                a_stage(jj)
                b_stage(jj - 1)
            for j in range(NGRP):
                c_conv(j)
                c_post(j)
                if j + 8 < NGRP:
                    a_stage(j + 8)
                if j + 7 < NGRP:
                    b_stage(j + 7)

    _split_multi_waits(nc)
    return nc


def _host_prep(adj, gcn_w, gcn_b, conv_w, conv_b, bn_gamma, bn_beta, bn_mean, bn_var):
    adj = np.asarray(adj, np.float64)
    norm = adj.sum(axis=1) ** -0.5
    an = (norm[:, None] * adj * norm[None, :]).astype(np.float32)
    a10 = np.kron(np.eye(10, dtype=np.float32), an)            # [250,250]
    bdeo = np.stack([a10[0::2, :], a10[1::2, :]], axis=1).astype(BF16)  # [125,2,250]

    gw1 = np.asarray(gcn_w, np.float32)
    gw = np.concatenate([gw1, gw1], axis=0).astype(BF16)       # [128, 64] dup
    gb = np.asarray(gcn_b, np.float32)[:, None]                # [64,1]

    cw = np.asarray(conv_w, np.float32)[:, :, :, 0]            # [cout, cin, KT]
    wp2 = np.zeros((2 * C, 5, 2 * C), np.float32)
    for jj in range(5):
        wp2[0:C, jj, 0:C] = cw[:, :, 2 * jj].T
        wp2[0:C, jj, C:2 * C] = cw[:, :, 2 * jj].T
        if 2 * jj + 1 < KT:
            wp2[C:2 * C, jj, 0:C] = cw[:, :, 2 * jj + 1].T
            wp2[C:2 * C, jj, C:2 * C] = cw[:, :, 2 * jj + 1].T
    wp2 = wp2.astype(BF16)

    s = np.asarray(bn_gamma, np.float32) / np.sqrt(np.asarray(bn_var, np.float32) + BN_EPS)
    d = (np.asarray(conv_b, np.float32) - np.asarray(bn_mean, np.float32)) * s \
        + np.asarray(bn_beta, np.float32)
    bns2 = np.tile(s, 2)[:, None].astype(np.float32)
    bnd2 = np.tile(d, 2)[:, None].astype(np.float32)
    ident = np.eye(2 * C, dtype=BF16)
    return dict(bdeo=bdeo, gw=gw, gb=gb, wp2=wp2, bns2=bns2, bnd2=bnd2, ident=ident)


def kernel(h, adj, gcn_w, gcn_b, conv_w, conv_b, bn_gamma, bn_beta, bn_mean, bn_var):
    from concourse.bass_utils import run_bass_kernel_spmd

    h = np.ascontiguousarray(np.asarray(h, np.float32))
    weights = _host_prep(adj, gcn_w, gcn_b, conv_w, conv_b,
                         bn_gamma, bn_beta, bn_mean, bn_var)

    if "nc" not in _CACHE:
        _CACHE["nc"] = _build_program()
    nc = _CACHE["nc"]

    in_maps = []
    for i in range(NCORES):
        m = dict(weights)
        m["h"] = h[i * LPC:(i + 1) * LPC]
        in_maps.append(m)

    res = run_bass_kernel_spmd(nc, in_maps, core_ids=list(range(NCORES)))
    out = np.concatenate([res.results[i]["out"] for i in range(NCORES)], axis=0)
    return out.reshape(N, M, T, V, C)


# revision 38
# speedup vs baseline: 1.4559x; 1.0168x over previous
"""GTCN block (GCN -> temporal conv -> BN -> ReLU -> residual) on 8 trn2 cores.

Sharding: data-parallel over nodes. Each core takes 30000 nodes = 4 complete
(n, m) skeleton samples; adjacency / GCN / TCN params replicated.

Per-core dataflow (channel-major through the middle, node-PAIR major at the
edges so every HBM descriptor is a 512B line-rate run):
  hres [125p x 120pg x (2,64)] bf16: partition x of pair-group g2 holds nodes
        (250*g2+2x, +1) -- feeds both the aggregation and the residual.
  MM-A : even/odd split blockdiag adjacency, 2 accumulating matmuls per
         250-node pair-group -> (A~ h)^T [64c, 250n] channel-major psum
  MM-B : lhsT=W[64,64], rhs=(A~ h)^T bf16 -> y^T [64, 500]
  ACT  : relu(y+b) -> zT[0:64, t]; SBUF->SBUF DMA -> zT[64:128, t-25]
  conv : 5 accumulating matmuls, lhsT=[W2j;W2j+1]x2 dup [128,128], rhs=zT win
  ACT  : relu(s*conv+d), stride-2 reads (even/odd nodes) -> vt [128, 250]
  MM-T : PE transpose vt[:,125q:+125] -> pst[125 pair, 128=(2 nodes x 64c)]
  DVE  : pst + hres -> og staging; one SWDGE out-DMA per 4 groups (512B descs)
"""

import numpy as np
import ml_dtypes

N, M, T, V, C = 16, 2, 300, 25, 64
KT, PAD = 9, 4
BN_EPS = 1e-5
NCORES = 8
LPC = N * M * T * V // NCORES   # 30000 nodes per core
BLK = 5 * V                     # 125 nodes per block (5 graphs)
PG = 2 * BLK                    # 250 nodes per pair-group (10 graphs)
NPG = LPC // PG                 # 120 pair-groups per core
SAMP = T * V                    # 7500 nodes per (n,m) sample
SPC = LPC // SAMP               # 4 samples per core
PGS = SAMP // PG                # 30 pair-groups per sample
GRP = 500                       # nodes per group (2 pair-groups)
NGRP = LPC // GRP               # 60 groups per core
GPS = SAMP // GRP               # 15 groups per sample
ZPAD = PAD * V                  # 100 zero columns per sample edge
SW = ZPAD + SAMP + ZPAD         # 7700 padded sample width in zT
OB = 4                          # groups per out-DMA batch
BF16 = ml_dtypes.bfloat16

_CACHE = {}


def _split_multi_waits(nc, max_waits=1):
    """walrus in this toolchain rejects >1 sem wait per instruction
    ("Too many sync wait commands"); move excess waits onto preceding
    same-engine nops."""
    import concourse.mybir as mybir

    ctr = 0
    for f in nc.m.functions:
        for bb in f.blocks:
            out = []
            for ins in bb.instructions:
                si = ins.sync_info
                if si is not None and len(si.on_wait) > max_waits:
                    waits = list(si.on_wait)
                    keep = waits[len(waits) - max_waits:]
                    rest = waits[: len(waits) - max_waits]
                    for i in range(0, len(rest), max_waits):
                        ctr += 1
                        out.append(
                            mybir.InstNoOp(
                                name=f"I-wsplit-{ctr}",
                                engine=ins.engine,
                                sync_info=mybir.SyncInfo(
                                    on_wait=rest[i:i + max_waits], on_update=[]
                                ),
                            )
                        )
                    si.on_wait = keep
                out.append(ins)
            bb.instructions = out


def _build_program():
    import concourse.bass as bass
    import concourse.mybir as mybir
    import concourse.tile as tile

    f32 = mybir.dt.float32
    bf16 = mybir.dt.bfloat16
    RELU = mybir.ActivationFunctionType.Relu

    nc = bass.Bass()
    h_t = nc.declare_dram_parameter("h", [LPC, C], f32, isOutput=False)
    bd_t = nc.declare_dram_parameter("bdeo", [BLK, 2, PG], bf16, isOutput=False)
    gw_t = nc.declare_dram_parameter("gw", [2 * C, C], bf16, isOutput=False)
    gb_t = nc.declare_dram_parameter("gb", [C, 1], f32, isOutput=False)
    wp_t = nc.declare_dram_parameter("wp2", [2 * C, 5, 2 * C], bf16, isOutput=False)
    bns_t = nc.declare_dram_parameter("bns2", [2 * C, 1], f32, isOutput=False)
    bnd_t = nc.declare_dram_parameter("bnd2", [2 * C, 1], f32, isOutput=False)
    id_t = nc.declare_dram_parameter("ident", [2 * C, 2 * C], bf16, isOutput=False)
    out_t = nc.declare_dram_parameter("out", [LPC, C], f32, isOutput=True)

    # pair-major views: partition x <-> nodes (250*g2 + 2x, +1)
    h_rp = h_t[:, :].rearrange("(g2 p two) c -> p g2 (two c)", p=BLK, two=2)
    out_rp = out_t[:, :].rearrange("(g2 p two) c -> p g2 (two c)", p=BLK, two=2)

    with tile.TileContext(nc) as tc:
        import contextlib

        with contextlib.ExitStack() as ctx:
            persist = ctx.enter_context(tc.tile_pool(name="persist", bufs=1))
            abufp = ctx.enter_context(tc.tile_pool(name="abuf", bufs=8))
            vtp = ctx.enter_context(tc.tile_pool(name="vt", bufs=6))
            ogp = ctx.enter_context(tc.tile_pool(name="og", bufs=6))
            pa = ctx.enter_context(tc.tile_pool(name="pa", bufs=2, space="PSUM"))
            py = ctx.enter_context(tc.tile_pool(name="py", bufs=2, space="PSUM"))
            pc = ctx.enter_context(tc.tile_pool(name="pc", bufs=2, space="PSUM"))
            pt = ctx.enter_context(tc.tile_pool(name="pt", bufs=2, space="PSUM"))

            # ---- persistent state ----
            bd_sb = persist.tile([BLK, 2, PG], bf16)
            nc.sync.dma_start(out=bd_sb, in_=bd_t[:, :, :])
            # PE warm-up: throwaway matmuls on the adjacency while the first
            # h chunks land, so the HAM clock gate is at 2.4GHz (not the cold
            # 1.2GHz default) when the real pipeline starts
            psd = pa.tile([C, GRP], f32, name="psd", tag="psa")
            for w in range(12):
                nc.tensor.matmul(psd[:, 0:PG], lhsT=bd_sb[:, 0, 0:C],
                                 rhs=bd_sb[:, w % 2, :], start=True, stop=True)
            gwd_sb = persist.tile([2 * C, C], bf16)
            nc.sync.dma_start(out=gwd_sb, in_=gw_t[:, :])
            gb_sb = persist.tile([C, 1], f32)
            nc.sync.dma_start(out=gb_sb, in_=gb_t[:, :])
            wp_sb = persist.tile([2 * C, 5, 2 * C], bf16)
            nc.sync.dma_start(out=wp_sb, in_=wp_t[:, :, :])
            bns_sb = persist.tile([2 * C, 1], f32)
            nc.sync.dma_start(out=bns_sb, in_=bns_t[:, :])
            bnd_sb = persist.tile([2 * C, 1], f32)
            nc.sync.dma_start(out=bnd_sb, in_=bnd_t[:, :])
            id_sb = persist.tile([2 * C, 2 * C], bf16)
            nc.sync.dma_start(out=id_sb, in_=id_t[:, :])

            # h resident in bf16 pair-major (cast during SWDGE DMA); 512B/desc
            hres = persist.tile([BLK, NPG, 2, C], bf16)
            hchunks = [0, 1, 3, 7, 15, 25, 35, 45, 55, 65, 75, 85, 95, 105, NPG]
            for q in range(len(hchunks) - 1):
                nc.gpsimd.dma_start(
                    out=hres[:, hchunks[q]:hchunks[q + 1], :, :].rearrange(
                        "p g a b -> p g (a b)"),
                    in_=h_rp[:, hchunks[q]:hchunks[q + 1], :])

            zt = persist.tile([2 * C, SPC * SW], bf16)
            for s in range(SPC):
                b = s * SW
                nc.vector.memset(zt[:, b:b + ZPAD], 0.0)
                nc.vector.memset(zt[:, b + ZPAD + SAMP:b + SW], 0.0)
                nc.vector.memset(zt[64:128, b + ZPAD + SAMP - V:b + ZPAD + SAMP], 0.0)

            # ---- pipeline over 500-node groups ----
            abufs = {}
            ogs = {}

            def a_stage(j):
                psa = pa.tile([C, GRP], f32)
                for i in range(2):
                    g2 = 2 * j + i
                    nc.tensor.matmul(
                        psa[:, i * PG:(i + 1) * PG],
                        lhsT=hres[:, g2, 0, :], rhs=bd_sb[:, 0, :],
                        start=True, stop=False,
                    )
                    nc.tensor.matmul(
                        psa[:, i * PG:(i + 1) * PG],
                        lhsT=hres[:, g2, 1, :], rhs=bd_sb[:, 1, :],
                        start=False, stop=True,
                    )
                ab = abufp.tile([C, GRP], bf16)
                nc.vector.tensor_copy(ab, psa)
                abufs[j] = ab

            def b_stage(j):
                s, l = divmod(j, GPS)
                zoff = s * SW + ZPAD + l * GRP
                psy = py.tile([C, GRP], f32)
                nc.tensor.matmul(psy, lhsT=gwd_sb[0:C, :], rhs=abufs.pop(j),
                                 start=True, stop=True)
                nc.scalar.activation(
                    out=zt[0:C, zoff:zoff + GRP], in_=psy,
                    func=RELU, bias=gb_sb,
                )
                # t+1-shifted copy for the bottom partition half: cross-partition,
                # so an SBUF->SBUF DMA (reads the relu'd top half). Issued from
                # ScalarE's HWDGE ring so it never queues behind bulk h/out DMAs.
                nc.scalar.dma_start(
                    out=zt[C:2 * C, zoff - V:zoff + GRP - V],
                    in_=zt[0:C, zoff:zoff + GRP],
                )

            pscs = {}

            def c_conv(j):
                s, l = divmod(j, GPS)
                zoff = s * SW + ZPAD + l * GRP
                psc = pc.tile([2 * C, GRP], f32, name="psc", tag="psc")
                for jj in range(5):
                    nc.tensor.matmul(
                        psc,
                        lhsT=wp_sb[:, jj, :],
                        rhs=zt[:, zoff + (2 * jj - 4) * V: zoff + (2 * jj - 4) * V + GRP],
                        start=(jj == 0), stop=(jj == 4),
                    )
                pscs[j] = psc

            def c_post(j):
                psc = pscs.pop(j)
                # even nodes -> vt top half, odd nodes -> vt bottom half
                # (psc bottom partition half duplicates the top, keeping the
                # stride-2 reads partition-aligned)
                vt = vtp.tile([2 * C, PG], bf16)
                pcv = psc.rearrange("p (x two) -> p x two", two=2)
                nc.scalar.activation(
                    out=vt[0:C, :], in_=pcv[0:C, :, 0],
                    func=RELU, bias=bnd_sb[0:C, :], scale=bns_sb[0:C, :],
                )
                nc.scalar.activation(
                    out=vt[C:2 * C, :], in_=pcv[C:2 * C, :, 1],
                    func=RELU, bias=bnd_sb[C:2 * C, :], scale=bns_sb[C:2 * C, :],
                )
                jb, jo = divmod(j, OB)
                if jo == 0:
                    ogs[jb] = ogp.tile([BLK, 2 * OB, 2 * C], f32, name="og", tag="og")
                og = ogs[jb]
                for q in range(2):
                    pst = pt.tile([BLK, 2 * C], bf16)
                    nc.tensor.transpose(pst, in_=vt[:, q * BLK:(q + 1) * BLK],
                                        identity=id_sb)
                    nc.vector.tensor_add(
                        og[:, jo * 2 + q, :],
                        pst,
                        hres[:, 2 * j + q, :, :].rearrange("p a b -> p (a b)"),
                    )
                last = jb == NGRP // OB - 1
                if last:
                    # flush per group at the end so the kernel-tail drain only
                    # waits on one tiny DMA
                    nc.gpsimd.dma_start(
                        out=out_rp[:, jb * 2 * OB + 2 * jo:jb * 2 * OB + 2 * jo + 2, :],
                        in_=og[:, 2 * jo:2 * jo + 2, :])
                    if jo == OB - 1:
                        ogs.pop(jb)
                elif jo == OB // 2 - 1:
                    # flush the first half-batch early
                    nc.gpsimd.dma_start(
                        out=out_rp[:, jb * 2 * OB:jb * 2 * OB + OB, :],
                        in_=og[:, 0:OB, :])
                elif jo == OB - 1:
                    nc.gpsimd.dma_start(
                        out=out_rp[:, jb * 2 * OB + OB:(jb + 1) * 2 * OB, :],
                        in_=ogs.pop(jb)[:, OB:2 * OB, :])

            a_stage(0)
            a_stage(1)
            b_stage(0)
            for jj in range(2, 8):
                a_stage(jj)
                b_stage(jj - 1)
            for j in range(NGRP):
                c_conv(j)
                c_post(j)
                if j + 8 < NGRP:
                    a_stage(j + 8)
                if j + 7 < NGRP:
                    b_stage(j + 7)

    _split_multi_waits(nc)
    return nc


def _host_prep(adj, gcn_w, gcn_b, conv_w, conv_b, bn_gamma, bn_beta, bn_mean, bn_var):
    adj = np.asarray(adj, np.float64)
    norm = adj.sum(axis=1) ** -0.5
    an = (norm[:, None] * adj * norm[None, :]).astype(np.float32)
    a10 = np.kron(np.eye(10, dtype=np.float32), an)            # [250,250]
    bdeo = np.stack([a10[0::2, :], a10[1::2, :]], axis=1).astype(BF16)  # [125,2,250]

    gw1 = np.asarray(gcn_w, np.float32)
    gw = np.concatenate([gw1, gw1], axis=0).astype(BF16)       # [128, 64] dup
    gb = np.asarray(gcn_b, np.float32)[:, None]                # [64,1]

    cw = np.asarray(conv_w, np.float32)[:, :, :, 0]            # [cout, cin, KT]
    wp2 = np.zeros((2 * C, 5, 2 * C), np.float32)
    for jj in range(5):
        wp2[0:C, jj, 0:C] = cw[:, :, 2 * jj].T
        wp2[0:C, jj, C:2 * C] = cw[:, :, 2 * jj].T
        if 2 * jj + 1 < KT:
            wp2[C:2 * C, jj, 0:C] = cw[:, :, 2 * jj + 1].T
            wp2[C:2 * C, jj, C:2 * C] = cw[:, :, 2 * jj + 1].T
    wp2 = wp2.astype(BF16)

    s = np.asarray(bn_gamma, np.float32) / np.sqrt(np.asarray(bn_var, np.float32) + BN_EPS)
    d = (np.asarray(conv_b, np.float32) - np.asarray(bn_mean, np.float32)) * s \
        + np.asarray(bn_beta, np.float32)
    bns2 = np.tile(s, 2)[:, None].astype(np.float32)
    bnd2 = np.tile(d, 2)[:, None].astype(np.float32)
    ident = np.eye(2 * C, dtype=BF16)
    return dict(bdeo=bdeo, gw=gw, gb=gb, wp2=wp2, bns2=bns2, bnd2=bnd2, ident=ident)


def kernel(h, adj, gcn_w, gcn_b, conv_w, conv_b, bn_gamma, bn_beta, bn_mean, bn_var):
    from concourse.bass_utils import run_bass_kernel_spmd

    h = np.ascontiguousarray(np.asarray(h, np.float32))
    weights = _host_prep(adj, gcn_w, gcn_b, conv_w, conv_b,
                         bn_gamma, bn_beta, bn_mean, bn_var)

    if "nc" not in _CACHE:
        _CACHE["nc"] = _build_program()
    nc = _CACHE["nc"]

    in_maps = []
    for i in range(NCORES):
        m = dict(weights)
        m["h"] = h[i * LPC:(i + 1) * LPC]
        in_maps.append(m)

    res = run_bass_kernel_spmd(nc, in_maps, core_ids=list(range(NCORES)))
    out = np.concatenate([res.results[i]["out"] for i in range(NCORES)], axis=0)
    return out.reshape(N, M, T, V, C)
